# revision 17
# baseline (speedup 1.0000x reference)
"""Trainium2 Bass kernel for BarrelShifterRight8.

Problem: X [N, 8] f32 bits (0/1, MSB-first), shift [N, 4] f32 bits
(MSB-first: shift[:,0]=by-8, [:,1]=by-4, [:,2]=by-2, [:,3]=by-1).
out[r, j] = X[r, j-k] for j>=k else 0, where k = packed shift amount
(>=8 -> all zeros).

Strategy (memory-bound; traffic fixed at 40 MiB/core = 24 in + 16 out):
  - Data-parallel shard rows across 8 NeuronCores, no communication.
  - DMA probes measured per-core 435 GB/s pure-read, 478 GB/s
    pure-write, but only ~370 GB/s with directions interleaved (HBM
    read/write turnaround). The shipped kernel (build_nc_phased)
    therefore PHASE-SEPARATES the DMA: all loads are issued before all
    stores on a single HWDGE ring (nc.sync), whose FIFO descriptor
    drain keeps the read phase pure, then the write phase pure
    (~93us floor vs ~113us interleaved).
  - X chunks land in a persistent 128KB/partition SBUF accumulator;
    per-chunk compute overlaps the read phase and writes IN PLACE over
    the consumed X chunk:
      pack:   V = row bits -> int via 3-level f32 Horner tree (DVE)
      k:      2-level tree over the 4 shift bits -> int32 (DVE)
      shift:  V' = V >> k (k>=8 zeroes V' for free since V < 256)
      unpack: out bit j = (V' >> (7-j)) & 1, strided int32 writes into
              a bitcast view of the chunk (DVE), then one in-place
              int32->f32 converting copy on ACT (own SBUF ports, fully
              parallel to DVE; walrus rejects Pool tensor ops and
              bitwise ops cannot cast, so this is the cheapest legal
              int->f32 path).
  - The original streaming kernel is kept as build_nc (BEST_MODE knob).
"""

import numpy as np

N_FULL = 4194304
N_CORES = 8
R_PER_CORE = N_FULL // N_CORES  # 524288 rows per core
P = 128
ROWS_PP = 512  # rows per partition per tile -> [128, 4096] f32 X tiles


def build_nc(rows: int, rows_pp: int = ROWS_PP, repeat: int = 1,
             io_bufs: int = 3, wk_bufs: int = 2, store_ring: str = "scalar",
             shift_ring: str = "sync", unpack: str = "copy",
             unpack_engines: str = "vvvvvvvv"):
    import concourse.mybir as mybir
    from concourse import bacc
    from concourse.tile import TileContext

    f32 = mybir.dt.float32
    i32 = mybir.dt.int32
    A = mybir.AluOpType

    rows_per_tile = P * rows_pp
    assert rows % rows_per_tile == 0, (rows, rows_per_tile)
    ntiles = rows // rows_per_tile
    T = rows_pp
    W = T * 8
    WS = T * 4

    nc = bacc.Bacc(None, target_bir_lowering=False)
    X = nc.declare_dram_parameter("X", [rows, 8], f32, isOutput=False)
    S = nc.declare_dram_parameter("shift", [rows, 4], f32, isOutput=False)
    O = nc.declare_dram_parameter("out", [rows, 8], f32, isOutput=True)

    Xv = X[:].rearrange("(n p r) c -> n p (r c)", p=P, r=rows_pp)
    Sv = S[:].rearrange("(n p r) c -> n p (r c)", p=P, r=rows_pp)
    Ov = O[:].rearrange("(n p r) c -> n p (r c)", p=P, r=rows_pp)

    rings = {"sync": nc.sync, "scalar": nc.scalar}
    with TileContext(nc) as tc:
        with (
            tc.tile_pool(name="io", bufs=io_bufs) as io,
            tc.tile_pool(name="work", bufs=wk_bufs) as wk,
        ):
            for it in range(ntiles * repeat):
                i = it % ntiles
                store_eng = rings[store_ring]
                xt = io.tile([P, W], f32, tag="xt")
                st = io.tile([P, WS], f32, tag="st")
                rings["sync"].dma_start(xt[:], Xv[i])
                rings[shift_ring].dma_start(st[:], Sv[i])

                # pack tree: V = sum_j 2^(7-j) * x[j], exact in f32
                x2 = xt[:].rearrange("p (r c) -> p r c", c=2)
                a = wk.tile([P, 4 * T], f32, tag="a")
                a1 = a[:].rearrange("p (r c) -> p r c", c=1)
                nc.vector.scalar_tensor_tensor(
                    a1, x2[:, :, 0:1], 2.0, x2[:, :, 1:2], A.mult, A.add
                )
                a2 = a[:].rearrange("p (r c) -> p r c", c=2)
                b = wk.tile([P, 2 * T], f32, tag="b")
                b1 = b[:].rearrange("p (r c) -> p r c", c=1)
                nc.vector.scalar_tensor_tensor(
                    b1, a2[:, :, 0:1], 4.0, a2[:, :, 1:2], A.mult, A.add
                )
                b2 = b[:].rearrange("p (r c) -> p r c", c=2)
                V = wk.tile([P, T], i32, tag="V")
                V1 = V[:].rearrange("p (r c) -> p r c", c=1)
                nc.vector.scalar_tensor_tensor(
                    V1, b2[:, :, 0:1], 16.0, b2[:, :, 1:2], A.mult, A.add
                )

                # k tree: k = 8*s0 + 4*s1 + 2*s2 + s3
                s2v = st[:].rearrange("p (r c) -> p r c", c=2)
                ka = wk.tile([P, 2 * T], f32, tag="ka")
                ka1 = ka[:].rearrange("p (r c) -> p r c", c=1)
                nc.vector.scalar_tensor_tensor(
                    ka1, s2v[:, :, 0:1], 2.0, s2v[:, :, 1:2], A.mult, A.add
                )
                ka2 = ka[:].rearrange("p (r c) -> p r c", c=2)
                k = wk.tile([P, T], i32, tag="k")
                k1 = k[:].rearrange("p (r c) -> p r c", c=1)
                nc.vector.scalar_tensor_tensor(
                    k1, ka2[:, :, 0:1], 4.0, ka2[:, :, 1:2], A.mult, A.add
                )

                # V' = V >> k (int32; k in 0..15, V < 256 -> k>=8 gives 0)
                Vs = wk.tile([P, T], i32, tag="Vs")
                nc.vector.tensor_tensor(
                    Vs[:], V[:], k[:], A.logical_shift_right
                )

                Vs1 = Vs[:].rearrange("p (r c) -> p r c", c=1)
                if unpack == "cmp":
                    # fused unpack+cast: out[:,j] = (Vs & 1<<(7-j)) != 0,
                    # comparison writes f32 0/1 directly -> no int tile, no
                    # converting copy. All DVE ops then run single-port
                    # modes, so Pool (gpsimd) offload never contends.
                    ot = wk.tile([P, W], f32, tag="ot")
                    ot3 = ot[:].rearrange("p (r c) -> p r c", c=8)
                    engs = {"v": nc.vector, "g": nc.gpsimd}
                    for j in range(8):
                        engs[unpack_engines[j]].tensor_scalar(
                            ot3[:, :, j : j + 1], Vs1, 1 << (7 - j), 0,
                            A.bitwise_and, A.not_equal,
                        )
                else:
                    # unpack bits -> int32 tile (bitVec ops can't cast), then
                    # one converting copy (2x_2p) to the f32 output tile
                    oi = wk.tile([P, W], i32, tag="oi")
                    oi3 = oi[:].rearrange("p (r c) -> p r c", c=8)
                    for j in range(8):
                        nc.vector.tensor_scalar(
                            oi3[:, :, j : j + 1], Vs1, 7 - j, 1,
                            A.logical_shift_right, A.bitwise_and,
                        )
                    ot = wk.tile([P, W], f32, tag="ot")
                    nc.vector.tensor_copy(ot[:], oi[:])

                store_eng.dma_start(Ov[i], ot[:])
    nc.compile()
    return nc


def build_nc_phased(rows: int, rows_pp: int = ROWS_PP, repeat: int = 1,
                    s_bufs: int = 4, wk_bufs: int = 2,
                    convert: str = "act", bt_bufs: int = 4):
    """Phase-separated DMA kernel: all loads, then all stores, on ONE
    HWDGE ring (sync). Ring FIFO keeps the read phase pure, then the
    write phase pure; DMA probes measured 435 GB/s pure-read and 478
    pure-write vs 370 GB/s mixed, so phasing saves ~17% wall clock.

    X chunks land in a persistent 128KB/partition accumulator (XO);
    compute runs per chunk during the read phase: DVE pack-shift, then
    DVE unpacks bits as int32 IN PLACE over the consumed X chunk (via a
    bitcast view), and ACT (own SBUF ports, runs parallel to DVE)
    converts the chunk int32 -> f32 in place. Stores then stream XO.
    convert: 'act' (scalar engine, parallel) or 'dve' (fallback).
    """
    import concourse.mybir as mybir
    from concourse import bacc
    from concourse.tile import TileContext

    f32 = mybir.dt.float32
    i32 = mybir.dt.int32
    A = mybir.AluOpType

    rows_per_tile = P * rows_pp
    assert rows % rows_per_tile == 0, (rows, rows_per_tile)
    ntiles = rows // rows_per_tile
    T = rows_pp
    W = T * 8
    WS = T * 4

    nc = bacc.Bacc(None, target_bir_lowering=False)
    X = nc.declare_dram_parameter("X", [rows, 8], f32, isOutput=False)
    S = nc.declare_dram_parameter("shift", [rows, 4], f32, isOutput=False)
    O = nc.declare_dram_parameter("out", [rows, 8], f32, isOutput=True)

    Xv = X[:].rearrange("(n p r) c -> n p (r c)", p=P, r=rows_pp)
    Sv = S[:].rearrange("(n p r) c -> n p (r c)", p=P, r=rows_pp)
    Ov = O[:].rearrange("(n p r) c -> n p (r c)", p=P, r=rows_pp)

    with TileContext(nc) as tc:
        with (
            tc.tile_pool(name="xo", bufs=1) as xo_pool,
            tc.tile_pool(name="sio", bufs=s_bufs) as sio,
            tc.tile_pool(name="wk", bufs=wk_bufs) as wk,
            tc.tile_pool(name="bt", bufs=bt_bufs) as bt,
        ):
            XO = xo_pool.tile([P, ntiles * W], f32, tag="XO")
            for _rep in range(repeat):
                sts = []
                for c in range(ntiles):
                    nc.sync.dma_start(XO[:, c * W : (c + 1) * W], Xv[c])
                    st = sio.tile([P, WS], f32, tag="st")
                    nc.sync.dma_start(st[:], Sv[c])
                    sts.append(st)

                for c in range(ntiles):
                    xt = XO[:, c * W : (c + 1) * W]
                    st = sts[c]
                    # pack tree: V = sum_j 2^(7-j) * x[j], exact in f32
                    x2 = xt.rearrange("p (r c) -> p r c", c=2)
                    a = wk.tile([P, 4 * T], f32, tag="a")
                    a1 = a[:].rearrange("p (r c) -> p r c", c=1)
                    nc.vector.scalar_tensor_tensor(
                        a1, x2[:, :, 0:1], 2.0, x2[:, :, 1:2], A.mult, A.add
                    )
                    a2 = a[:].rearrange("p (r c) -> p r c", c=2)
                    b = wk.tile([P, 2 * T], f32, tag="b")
                    b1 = b[:].rearrange("p (r c) -> p r c", c=1)
                    nc.vector.scalar_tensor_tensor(
                        b1, a2[:, :, 0:1], 4.0, a2[:, :, 1:2], A.mult, A.add
                    )
                    b2 = b[:].rearrange("p (r c) -> p r c", c=2)
                    V = wk.tile([P, T], i32, tag="V")
                    V1 = V[:].rearrange("p (r c) -> p r c", c=1)
                    nc.vector.scalar_tensor_tensor(
                        V1, b2[:, :, 0:1], 16.0, b2[:, :, 1:2], A.mult, A.add
                    )

                    # k tree: k = 8*s0 + 4*s1 + 2*s2 + s3
                    s2v = st[:].rearrange("p (r c) -> p r c", c=2)
                    ka = wk.tile([P, 2 * T], f32, tag="ka")
                    ka1 = ka[:].rearrange("p (r c) -> p r c", c=1)
                    nc.vector.scalar_tensor_tensor(
                        ka1, s2v[:, :, 0:1], 2.0, s2v[:, :, 1:2], A.mult, A.add
                    )
                    ka2 = ka[:].rearrange("p (r c) -> p r c", c=2)
                    k = wk.tile([P, T], i32, tag="k")
                    k1 = k[:].rearrange("p (r c) -> p r c", c=1)
                    nc.vector.scalar_tensor_tensor(
                        k1, ka2[:, :, 0:1], 4.0, ka2[:, :, 1:2], A.mult, A.add
                    )

                    # V' = V >> k (k>=8 zeroes V' for free since V < 256)
                    Vs = wk.tile([P, T], i32, tag="Vs")
                    nc.vector.tensor_tensor(
                        Vs[:], V[:], k[:], A.logical_shift_right
                    )

                    ot3 = xt.rearrange("p (r c) -> p r c", c=8)
                    # hybrid: cheap strided unpack + one whole-chunk ACT
                    # convert for early chunks (fewest cross-engine
                    # syncs); dense bit-plane unpack (2x DVE mode) + ACT
                    # interleave for the LAST chunks, whose DVE
                    # completion time gates the final stores.
                    mode = convert
                    if convert == "hybrid":
                        mode = "dense_act" if c >= ntiles // 2 else "act"
                    if mode == "dense_act":
                        # DVE unpacks each bit into a DENSE int32 plane
                        # (single-src dense -> 2x mode, half the strided
                        # cost); ACT (own SBUF ports, idle otherwise)
                        # interleaves each plane into the output column
                        # with the int32->f32 cast. DVE/chunk drops
                        # ~11us -> ~8.9us, keeping the last store on the
                        # pure-write schedule.
                        for j in range(8):
                            pl = bt.tile([P, T], i32, tag="pl")
                            nc.vector.tensor_scalar(
                                pl[:], Vs[:], 7 - j, 1,
                                A.logical_shift_right, A.bitwise_and,
                            )
                            pl1 = pl[:].rearrange("p (r c) -> p r c", c=1)
                            nc.scalar.copy(ot3[:, :, j : j + 1], pl1)
                    else:
                        # unpack bits as int32 IN PLACE over the consumed
                        # X chunk (bitcast view), then convert int32 ->
                        # f32 in place on ACT (or DVE as fallback)
                        xi = xt.bitcast(i32)
                        oi3 = xi.rearrange("p (r c) -> p r c", c=8)
                        Vs1 = Vs[:].rearrange("p (r c) -> p r c", c=1)
                        for j in range(8):
                            nc.vector.tensor_scalar(
                                oi3[:, :, j : j + 1], Vs1, 7 - j, 1,
                                A.logical_shift_right, A.bitwise_and,
                            )
                        if mode == "act":
                            nc.scalar.copy(xt, xi)
                        else:
                            nc.vector.tensor_copy(xt, xi)

                for c in range(ntiles):
                    nc.sync.dma_start(Ov[c], XO[:, c * W : (c + 1) * W])
    nc.compile()
    return nc


_NC_CACHE: dict = {}

# winning config: phase-separated DMA + dense bit-plane unpack with ACT
# interleave (see build_nc_phased docstring). Flip BEST_MODE to "stream"
# to fall back to the original streaming kernel.
BEST_MODE = "phased"
BEST_CFG = {"convert": "hybrid", "s_bufs": 5, "wk_bufs": 1}


def build_best(repeat: int = 1):
    if BEST_MODE == "phased":
        return build_nc_phased(R_PER_CORE, ROWS_PP, repeat=repeat, **BEST_CFG)
    return build_nc(R_PER_CORE, ROWS_PP, repeat=repeat)


def _get_nc():
    if "nc" not in _NC_CACHE:
        _NC_CACHE["nc"] = build_best()
    return _NC_CACHE["nc"]


def kernel(X: np.ndarray, shift: np.ndarray) -> np.ndarray:
    from concourse.bass_utils import run_bass_kernel_spmd

    X = np.ascontiguousarray(X, dtype=np.float32)
    shift = np.ascontiguousarray(shift, dtype=np.float32)
    assert X.shape == (N_FULL, 8) and shift.shape == (N_FULL, 4)

    nc = _get_nc()
    R = R_PER_CORE
    in_maps = [
        {
            "X": X[i * R : (i + 1) * R],
            "shift": shift[i * R : (i + 1) * R],
        }
        for i in range(N_CORES)
    ]
    res = run_bass_kernel_spmd(nc, in_maps, core_ids=list(range(N_CORES)))
    return np.concatenate([r["out"] for r in res.results], axis=0)

